# revision 23
# baseline (speedup 1.0000x reference)
"""GCN message-passing kernel (nn_Encoder_953482739902) for 8 TRN2 NeuronCores.

Computation (PyG GCNConv + mish):
    deg  = in-degree(col) + 1 (self-loops)
    dinv = deg^-1/2
    agg[t] = sum_{e: col(e)=t} dinv[row_e] * x[row_e]   (self-loops included)
    out  = mish(dinv[t] * (agg @ W) + b)

Distribution: targets (and output rows) sharded 8 ways; each core owns 12500
targets and the edges pointing at them (edge partition by target node). Every
core receives the full bf16 feature table in its HBM.

Per-core device pipeline (all floating-point math on device):
  - Prologue: the feature table streams through SBUF once (4 chunks of 25088
    rows); each chunk is row-scaled by dinv(src) on the ACT engine (dinv from
    integer degrees via Ln/Exp on device) and written back to a DRAM xs
    table in bf16.
  - Edge messages are dma_gather'ed from xs (256B rows) into edge-major
    [128e, su, 128f] tiles and scattered into 256-target windows via one-hot
    matmuls (bf16), accumulating in PSUM per (chunk, window) slot, then
    added into 50 persistent SBUF window accumulators. The one-hot rhs
    tiles are pure 0/1 index encodings, prebuilt host-side and streamed
    from HBM (keeps the Vector engine off the SBUF ports the Q7 descriptor
    generator saturates).
  - Per-core windows are load-balanced (serpentine assignment of targets by
    degree) to minimize cross-core slot padding; outputs are unpermuted on
    the host.
  - Finalize per window: agg @ W (bf16 matmul), scale by dinv(t), exact
    mish via z * a / (a + 2) with a = u^2 + 2u, u = exp(z).

Host side does index work only: bincount (degree), sorting/partitioning,
one-hot packing, int casts, layout packing.
"""

import numpy as np

N_NODES = 100000
IN_CH = 128
N_CORES = 8
TPC = 12500              # targets per core
WIN = 256                # window width (positions); 250 real targets + pad
TPW = 250                # real targets per window
NW = TPC // TPW          # 50 windows
TPAD = NW * WIN          # 12800 output rows per core (window-major positions)
NCHUNK = 4
CHUNK = 25088            # rows per source chunk (196 blocks of 128)
NBLK = CHUNK // 128      # 196
NROWS_PAD = NCHUNK * CHUNK
GU_MAX = 96              # subtiles (128 idx each) per dma_gather unit
OB_BATCH = 16            # one-hot subtiles per HBM load
NQ = 1                   # SWDGE queues


def _build_schedule(row, col):
    """Partition edges; balance windows per core; return host arrays.

    row/col: int64 [E_tot] including self-loops.
    SPMD: slot sizes (s_slot) are shared across cores (max over cores).
    """
    # strided target sharding (core = t % 8) so each core's self-loops
    # spread uniformly over source chunks (keeps slot sizes balanced)
    core = col % N_CORES
    ch = row // CHUNK
    srcloc = (row - ch * CHUNK).astype(np.int64)

    # per-target in-edge counts -> serpentine window assignment per core
    tcnt = np.bincount(col, minlength=N_NODES)
    wassign = np.empty(N_NODES, np.int64)
    wpos = np.empty(N_NODES, np.int64)
    for cc in range(N_CORES):
        tgts = np.arange(cc, N_NODES, N_CORES)
        rank = np.argsort(-tcnt[tgts], kind="stable")
        snake = rank.reshape(TPW, NW).copy()   # row k = rank[k*NW:(k+1)*NW]
        snake[1::2] = snake[1::2, ::-1]        # serpentine
        # local target snake[k, j] -> window j, position k
        wa = np.empty(TPC, np.int64)
        wp = np.empty(TPC, np.int64)
        kk, jj = np.divmod(np.arange(TPW * NW), NW)
        wa[snake.ravel()] = jj
        wp[snake.ravel()] = kk
        wassign[tgts] = wa
        wpos[tgts] = wp

    w = wassign[col]
    tl = wpos[col]                             # window-relative position

    key = ((core * NCHUNK + ch) * NW + w).astype(np.int64)
    counts = np.bincount(key, minlength=N_CORES * NCHUNK * NW).reshape(
        N_CORES, NCHUNK, NW
    )
    s_slot = -(-counts.max(axis=0) // 128)     # [NCHUNK, NW] subtiles
    nsub = int(s_slot.sum())

    slots = []                                 # (chunk, window, sub_base, n)
    sub_base = np.zeros((NCHUNK, NW), np.int64)
    n = 0
    for c in range(NCHUNK):
        for ww in range(NW):
            sub_base[c, ww] = n
            if s_slot[c, ww] > 0:
                slots.append((c, ww, n, int(s_slot[c, ww])))
            n += int(s_slot[c, ww])
    assert n == nsub

    units = []                                 # (chunk, sub_base, su, queue)
    qn = 0
    for c in range(NCHUNK):
        lo = int(sub_base[c, 0])
        hi = lo + int(s_slot[c].sum())
        off = lo
        while off < hi:
            su = min(GU_MAX, hi - off)
            units.append((c, off, su, qn % NQ))
            qn += 1
            off += su

    order = np.lexsort((row, w, ch, core))
    so = srcloc[order]
    tlo = tl[order]
    cum = np.zeros((N_CORES, NCHUNK, NW + 1), np.int64)
    cum[:, :, 1:] = np.cumsum(counts, axis=2)
    ccum = np.zeros((N_CORES, NCHUNK + 1), np.int64)
    ccum[:, 1:] = np.cumsum(counts.sum(axis=2), axis=1)
    gcum = np.zeros(N_CORES + 1, np.int64)
    gcum[1:] = np.cumsum(counts.sum(axis=(1, 2)))

    idx16 = np.zeros((N_CORES, 128, nsub * 8), np.int16)
    tloc = np.full((N_CORES, 128, nsub), -1, np.int32)

    for cc in range(N_CORES):
        for c in range(NCHUNK):
            for ww in range(NW):
                a = gcum[cc] + ccum[cc, c] + cum[cc, c, ww]
                b = gcum[cc] + ccum[cc, c] + cum[cc, c, ww + 1]
                cnt = b - a
                if cnt == 0:
                    continue
                sb = int(sub_base[c, ww])
                npad = int(s_slot[c, ww]) * 128
                sv = np.zeros(npad, np.int16)
                tv = np.full(npad, -1, np.int32)
                sv[:cnt] = so[a:b].astype(np.int16)
                tv[:cnt] = tlo[a:b].astype(np.int32)
                wrapped = sv.reshape(-1, 16).T       # [16, npad/16]
                idx16[cc, :, sb * 8: sb * 8 + npad // 16] = np.tile(
                    wrapped, (8, 1))
                tloc[cc, :, sb: sb + npad // 128] = tv.reshape(-1, 128).T

    # output permutation: per-core window-major position -> global target id
    tperm = np.full((N_CORES, TPAD), -1, np.int64)
    for cc in range(N_CORES):
        tgts = np.arange(cc, N_NODES, N_CORES)
        pos = wassign[tgts] * WIN + wpos[tgts]
        tperm[cc, pos] = tgts

    return idx16, tloc, slots, units, nsub, tperm


def _fix_act_table_loads(nc):
    """All activations used (Ln, Exp, Copy-family) live in the single
    'natural_log_exp_and_others' table set; retarget every load there and
    drop redundant repeats (keep the first load per basic block)."""
    import concourse.mybir as mybir
    from concourse.hw_specs import get_activation_tables

    tables = get_activation_tables(nc.m.arch)
    names = list(tables.keys())
    target = "natural_log_exp_and_others"
    target_id = names.index(target)
    allowed = tables[target]
    for f in nc.m.functions:
        for blk in f.blocks:
            insts = blk.instructions
            for inst in insts:
                if isinstance(inst, mybir.InstActivation):
                    assert inst.func in allowed, inst.func
            kept = []
            seen_load = False
            for inst in insts:
                if isinstance(inst, mybir.InstLoadActFuncSet):
                    si = inst.sync_info
                    has_sync = si is not None and (si.on_wait or si.on_update)
                    if seen_load and not has_sync:
                        continue
                    inst.act_func_set_id = target_id
                    seen_load = True
                kept.append(inst)
            if len(kept) != len(insts):
                insts[:] = kept


def _build_bass(slots, units, nsub):
    import concourse.bacc as bacc
    import concourse.mybir as mybir
    from concourse.tile import TileContext

    AF = mybir.ActivationFunctionType
    OP = mybir.AluOpType
    BF = mybir.dt.bfloat16
    F32 = mybir.dt.float32

    nc = bacc.Bacc("TRN2", target_bir_lowering=False, debug=False,
                   num_devices=N_CORES, num_swdge_queues=NQ,
                   dynamic_dma_scratch_size=32768)
    _orig_compile = nc.compile

    def _compile_with_fix():
        _orig_compile()
        _fix_act_table_loads(nc)

    nc.compile = _compile_with_fix

    xbf_d = nc.dram_tensor("xbf", [NROWS_PAD, IN_CH], BF, kind="ExternalInput")
    idx_d = nc.dram_tensor("idx16", [128, nsub * 8], mybir.dt.int16,
                           kind="ExternalInput")
    ob_d = nc.dram_tensor("obig", [128, nsub * WIN], mybir.dt.float8e4,
                          kind="ExternalInput")
    w_d = nc.dram_tensor("Wb", [IN_CH, IN_CH], BF, kind="ExternalInput")
    degt_d = nc.dram_tensor("degt", [128, TPAD // 128], F32,
                            kind="ExternalInput")
    degs_d = nc.dram_tensor("degs", [128, NROWS_PAD // 128], F32,
                            kind="ExternalInput")
    out_d = nc.dram_tensor("out", [TPAD, IN_CH], F32, kind="ExternalOutput")

    units_by_c = [[] for _ in range(NCHUNK)]
    for u in units:
        units_by_c[u[0]].append(u)
    slots_by_c = [[] for _ in range(NCHUNK)]
    for s in slots:
        slots_by_c[s[0]].append(s)

    with TileContext(nc) as tc:
        with (
            tc.tile_pool(name="const", bufs=1) as cp,
            tc.tile_pool(name="chunk", bufs=1) as chp,
            tc.tile_pool(name="idxp", bufs=2) as ixp,
            tc.tile_pool(name="acc", bufs=1) as ap_,
            tc.tile_pool(name="gt", bufs=2) as gp,
            tc.tile_pool(name="obuf", bufs=3) as op_,
            tc.tile_pool(name="fin", bufs=2) as fp,
            tc.tile_pool(name="xs", bufs=1, space="DRAM") as xsp,
            tc.tile_pool(name="psw", bufs=3, space="PSUM") as pwp,
            tc.tile_pool(name="ps2", bufs=2, space="PSUM") as p2p,
        ):
            wsb = cp.tile([IN_CH, IN_CH], BF)
            nc.sync.dma_start(out=wsb[:], in_=w_d[:])
            degt = cp.tile([128, TPAD // 128], F32)
            nc.sync.dma_start(out=degt[:], in_=degt_d[:])
            degs = cp.tile([128, NROWS_PAD // 128], F32)
            nc.sync.dma_start(out=degs[:], in_=degs_d[:])

            # dinv = exp(-0.5 ln(deg)) for targets and sources
            lnt = cp.tile([128, TPAD // 128], F32)
            nc.scalar.activation(lnt[:], degt[:], AF.Ln)
            dinvt = cp.tile([128, TPAD // 128], F32)
            nc.scalar.activation(dinvt[:], lnt[:], AF.Exp, scale=-0.5)
            lns = cp.tile([128, NROWS_PAD // 128], F32)
            nc.scalar.activation(lns[:], degs[:], AF.Ln)
            dinvs = cp.tile([128, NROWS_PAD // 128], F32)
            nc.scalar.activation(dinvs[:], lns[:], AF.Exp, scale=-0.5)

            accs = [ap_.tile([128, WIN], BF, tag=f"acc{w}", name=f"acc{w}")
                    for w in range(NW)]
            touched = {s[1] for s in slots}
            for w in range(NW):
                if w not in touched:
                    nc.vector.memset(accs[w][:], 0.0)
            first_done = [False] * NW

            last_chunk_of = {}
            for (c_, ww_, _, _) in slots:
                last_chunk_of[ww_] = max(last_chunk_of.get(ww_, -1), c_)
            finalized = set()

            def _finalize_window(w):
                # out[t,:] = mish(dinv_t * (agg[t,:] @ W))
                finalized.add(w)
                for sl in range(WIN // 128):
                    t0 = w * WIN + sl * 128
                    ps2 = p2p.tile([128, 128], F32, tag="ps2", space="PSUM",
                                   name=f"ps2_{w}_{sl}")
                    nc.tensor.matmul(
                        out=ps2[:], lhsT=accs[w][:, sl * 128:(sl + 1) * 128],
                        rhs=wsb[:], start=True, stop=True)
                    zt = fp.tile([128, 128], F32, tag="zt", name=f"zt{w}{sl}")
                    nc.scalar.activation(
                        zt[:], ps2[:], AF.Copy,
                        scale=dinvt[:, 2 * w + sl: 2 * w + sl + 1])
                    u = fp.tile([128, 128], F32, tag="u", name=f"u{w}{sl}")
                    nc.scalar.activation(u[:], zt[:], AF.Exp)
                    a1 = fp.tile([128, 128], F32, tag="a1", name=f"a1{w}{sl}")
                    nc.scalar.activation(a1[:], u[:], AF.Copy, bias=2.0)
                    a = fp.tile([128, 128], F32, tag="a", name=f"a{w}{sl}")
                    nc.vector.tensor_tensor(out=a[:], in0=a1[:], in1=u[:],
                                            op=OP.mult)
                    den = fp.tile([128, 128], F32, tag="den",
                                  name=f"den{w}{sl}")
                    nc.scalar.activation(den[:], a[:], AF.Copy, bias=2.0)
                    rden = fp.tile([128, 128], F32, tag="rden",
                                   name=f"rden{w}{sl}")
                    nc.vector.reciprocal_approx_fast(out=rden[:], in_=den[:])
                    mt = fp.tile([128, 128], F32, tag="mt", name=f"mt{w}{sl}")
                    nc.vector.tensor_tensor(out=mt[:], in0=a[:], in1=rden[:],
                                            op=OP.mult)
                    mz = fp.tile([128, 128], F32, tag="mz", name=f"mz{w}{sl}")
                    nc.vector.tensor_tensor(out=mz[:], in0=mt[:], in1=zt[:],
                                            op=OP.mult)
                    nc.sync.dma_start(out=out_d[t0:t0 + 128, :], in_=mz[:])

            # prologue: build the prescaled bf16 source table xs in DRAM
            xs = [xsp.tile([CHUNK, IN_CH], BF, tag=f"xs{c}", name=f"xs{c}",
                           space="DRAM") for c in range(NCHUNK)]
            HB = NBLK // 2
            for c in range(NCHUNK):
                chunkt = chp.tile([128, NBLK, IN_CH], BF, tag="chunk")
                for h in range(2):
                    b0, b1 = h * HB, (h + 1) * HB if h == 0 else NBLK
                    r0, r1 = c * CHUNK + b0 * 128, c * CHUNK + b1 * 128
                    src = xbf_d[r0:r1, :].rearrange("(b p) f -> p b f", p=128)
                    nc.sync.dma_start(out=chunkt[:, b0:b1, :], in_=src)
                    for b in range(b0, b1):
                        sc = dinvs[:, c * NBLK + b: c * NBLK + b + 1]
                        if b % 2 == 0:
                            nc.scalar.activation(
                                chunkt[:, b, :], chunkt[:, b, :], AF.Copy,
                                scale=sc)
                        else:
                            nc.vector.tensor_scalar(
                                out=chunkt[:, b, :], in0=chunkt[:, b, :],
                                scalar1=sc, scalar2=None, op0=OP.mult)
                    nc.sync.dma_start(
                        out=xs[c][b0 * 128:b1 * 128, :].rearrange(
                            "(b p) f -> p b f", p=128),
                        in_=chunkt[:, b0:b1, :])

            sub_base_c = {}
            off = 0
            for c in range(NCHUNK):
                nsub_c = sum(u[2] for u in units_by_c[c])
                sub_base_c[c] = (off, nsub_c)
                off += nsub_c

            for c in range(NCHUNK):
                base_c, nsub_c = sub_base_c[c]
                idxt = ixp.tile([128, nsub_c * 8], mybir.dt.int16, tag="idxt")
                nc.sync.dma_start(
                    out=idxt[:],
                    in_=idx_d[:, base_c * 8:(base_c + nsub_c) * 8])

                slot_iter = iter(slots_by_c[c])
                cur_slot = next(slot_iter)
                mm_in_slot = 0

                for (_, sb, su, qn) in units_by_c[c]:
                    gT = gp.tile([128, GU_MAX, 128], BF, tag="gt")
                    nc.gpsimd.dma_gather(
                        out_ap=gT[:, :su, :],
                        in_ap=xs[c][:, :],
                        idxs_ap=idxt[:, (sb - base_c) * 8:
                                     (sb - base_c + su) * 8],
                        num_idxs=su * 128,
                        num_idxs_reg=su * 128,
                        elem_size=IN_CH,
                        transpose=False,
                        single_packet=False,
                        queue_num=qn,
                    )
                    for k in range(su):
                        j = sb + k       # global subtile index
                        if k % OB_BATCH == 0:
                            nob = min(OB_BATCH, su - k)
                            ob = op_.tile([128, OB_BATCH * WIN], mybir.dt.float8e4,
                                          tag="ob")
                            nc.sync.dma_start(
                                out=ob[:, :nob * WIN],
                                in_=ob_d[:, j * WIN:(j + nob) * WIN])
                        while j >= cur_slot[2] + cur_slot[3]:
                            cur_slot = next(slot_iter)
                            mm_in_slot = 0
                        (_, ww, ssb, ssz) = cur_slot
                        if mm_in_slot == 0:
                            psw = pwp.tile([128, WIN], F32, tag="psw",
                                           space="PSUM")
                            cur_psw = psw
                        nc.tensor.matmul(
                            out=cur_psw[:], lhsT=gT[:, k, :],
                            rhs=ob[:, (k % OB_BATCH) * WIN:
                                   (k % OB_BATCH + 1) * WIN],
                            start=(mm_in_slot == 0),
                            stop=(mm_in_slot == ssz - 1))
                        mm_in_slot += 1
                        if mm_in_slot == ssz:
                            if not first_done[ww]:
                                first_done[ww] = True
                                nc.scalar.activation(accs[ww][:], cur_psw[:],
                                                     AF.Copy)
                            else:
                                nc.vector.tensor_tensor(
                                    out=accs[ww][:], in0=accs[ww][:],
                                    in1=cur_psw[:], op=OP.add)
                            if c == last_chunk_of[ww]:
                                _finalize_window(ww)

            for w in range(NW):
                if w not in finalized:
                    _finalize_window(w)
    nc.finalize()
    return nc


class _Runner:
    """PJRT runner (axon): jit once, device-resident inputs, reusable."""

    def __init__(self, nc):
        import jax
        import concourse.mybir as mybir
        from jax.sharding import Mesh, PartitionSpec
        from jax.experimental.shard_map import shard_map
        from concourse import bass2jax
        from concourse.bass2jax import _bass_exec_p, install_neuronx_cc_hook

        install_neuronx_cc_hook()
        self.nc = nc
        partition_name = (
            nc.partition_id_tensor.name if nc.partition_id_tensor else None
        )
        in_names, out_names, out_avals, zero_outs = [], [], [], []
        for alloc in nc.m.functions[0].allocations:
            if not isinstance(alloc, mybir.MemoryLocationSet):
                continue
            name = alloc.memorylocations[0].name
            if alloc.kind == "ExternalInput":
                if name != partition_name:
                    in_names.append(name)
            elif alloc.kind == "ExternalOutput":
                shape = tuple(alloc.tensor_shape)
                dtype = mybir.dt.np(alloc.dtype)
                out_names.append(name)
                out_avals.append(jax.core.ShapedArray(shape, dtype))
                zero_outs.append(np.zeros(shape, dtype))
        self.in_names, self.out_names = in_names, out_names
        all_in = list(in_names) + list(out_names)
        if partition_name is not None:
            all_in.append(partition_name)

        def _body(*args):
            operands = list(args)
            if partition_name is not None:
                operands.append(bass2jax.partition_id_tensor())
            return tuple(_bass_exec_p.bind(
                *operands,
                out_avals=tuple(out_avals),
                in_names=tuple(all_in),
                out_names=tuple(out_names),
                lowering_input_output_aliases=(),
                sim_require_finite=True,
                sim_require_nnan=True,
                nc=nc,
            ))

        devices = jax.devices()[:N_CORES]
        mesh = Mesh(np.asarray(devices), ("core",))
        n_in = len(in_names) + len(out_names)
        self.fn = jax.jit(
            shard_map(_body, mesh=mesh,
                      in_specs=(PartitionSpec("core"),) * n_in,
                      out_specs=(PartitionSpec("core"),) * len(out_names),
                      check_rep=False),
            keep_unused=True)
        self.zero_outs = zero_outs
        self.jax = jax

    def stage(self, in_maps):
        args = []
        for name in self.in_names:
            args.append(np.concatenate(
                [np.asarray(m[name]) for m in in_maps], axis=0))
        for z in self.zero_outs:
            args.append(np.concatenate([z] * N_CORES, axis=0))
        self._dev_args = [self.jax.device_put(a) for a in args]
        for a in self._dev_args:
            a.block_until_ready()

    def run(self):
        outs = self.fn(*self._dev_args)
        for o in outs:
            o.block_until_ready()
        return outs

    def results(self, outs):
        per_core = [dict() for _ in range(N_CORES)]
        for i, name in enumerate(self.out_names):
            arr = np.asarray(outs[i])
            for c, piece in enumerate(np.split(arr, N_CORES, axis=0)):
                per_core[c][name] = piece
        return per_core


_CACHE = {}


def _build_inputs(x, W, deg, idx16, tloc, tperm, nsub):
    import ml_dtypes

    N = x.shape[0]
    xbf = np.zeros((NROWS_PAD, IN_CH), ml_dtypes.bfloat16)
    xbf[:N] = x.astype(ml_dtypes.bfloat16)

    # deg value of node n at [n%128, n//128] (pad rows: 1)
    degs = np.ones(NROWS_PAD, np.float32)
    degs[:N] = deg.astype(np.float32)
    degs = degs.reshape(-1, 128).T.copy()        # [128, NROWS_PAD//128]

    Wb = W.astype(ml_dtypes.bfloat16)

    in_maps = []
    for c in range(N_CORES):
        # one-hot scatter tiles: [128, nsub*WIN], O[p, j*WIN + tloc] = 1
        ob = np.zeros((128, nsub * WIN), ml_dtypes.float8_e4m3)
        tl = tloc[c]                              # [128, nsub] int32
        p_, j_ = np.nonzero(tl >= 0)
        ob[p_, j_ * WIN + tl[p_, j_]] = 1

        dp = np.ones(TPAD, np.float32)
        valid = tperm[c] >= 0
        dp[valid] = deg[tperm[c][valid]].astype(np.float32)
        degt = dp.reshape(-1, 128).T.copy()

        in_maps.append({
            "xbf": xbf,
            "idx16": np.ascontiguousarray(idx16[c]),
            "obig": ob,
            "Wb": Wb,
            "degt": np.ascontiguousarray(degt),
            "degs": degs,
        })
    return in_maps


def _prepare(x, edge_index, W, b):
    x = np.asarray(x, dtype=np.float32)
    edge_index = np.asarray(edge_index)
    W = np.asarray(W, dtype=np.float32)
    b = np.asarray(b, dtype=np.float32)
    N = x.shape[0]
    assert N == N_NODES and x.shape[1] == IN_CH
    assert not np.any(b != 0), "bias path not implemented (b is zero here)"

    row = edge_index[0].astype(np.int64)
    col = edge_index[1].astype(np.int64)
    loops = np.arange(N, dtype=np.int64)
    row_all = np.concatenate([row, loops])
    col_all = np.concatenate([col, loops])

    deg = np.bincount(col_all, minlength=N).astype(np.int64)

    idx16, tloc, slots, units, nsub, tperm = _build_schedule(row_all, col_all)
    in_maps = _build_inputs(x, W, deg, idx16, tloc, tperm, nsub)

    key = (nsub, tuple(units), tuple(slots))
    if key not in _CACHE:
        nc = _build_bass(slots, units, nsub)
        runner = _Runner(nc)
        _CACHE.clear()
        _CACHE[key] = runner
    return _CACHE[key], in_maps, tperm


def kernel(x, edge_index, W, b):
    runner, in_maps, tperm = _prepare(x, edge_index, W, b)
    runner.stage(in_maps)
    outs = runner.run()
    res = runner.results(outs)
    full = np.empty((N_NODES, IN_CH), np.float32)
    for c in range(N_CORES):
        valid = tperm[c] >= 0
        full[tperm[c][valid]] = res[c]["out"][valid]
    return full


# revision 24
# speedup vs baseline: 1.2126x; 1.2126x over previous
"""GCN message-passing kernel (nn_Encoder_953482739902) for 8 TRN2 NeuronCores.

Computation (PyG GCNConv + mish):
    deg  = in-degree(col) + 1 (self-loops)
    dinv = deg^-1/2
    agg[t] = sum_{e: col(e)=t} dinv[row_e] * x[row_e]   (self-loops included)
    out  = mish(dinv[t] * (agg @ W) + b)

Distribution: targets (and output rows) sharded 8 ways; each core owns 12500
targets and the edges pointing at them (edge partition by target node). Every
core receives the full bf16 feature table in its HBM.

Per-core device pipeline (all floating-point math on device):
  - Prologue: the feature table streams through SBUF once (4 chunks of 25088
    rows); each chunk is row-scaled by dinv(src) on the ACT engine (dinv from
    integer degrees via Ln/Exp on device) and written back to a DRAM xs
    table in bf16.
  - Edge messages are dma_gather'ed from xs (256B rows) into edge-major
    [128e, su, 128f] tiles and scattered into 256-target windows via one-hot
    matmuls (bf16), accumulating in PSUM per (chunk, window) slot, then
    added into 50 persistent SBUF window accumulators. The one-hot rhs
    tiles are pure 0/1 index encodings, prebuilt host-side and streamed
    from HBM (keeps the Vector engine off the SBUF ports the Q7 descriptor
    generator saturates).
  - Per-core windows are load-balanced (serpentine assignment of targets by
    degree) to minimize cross-core slot padding; outputs are unpermuted on
    the host.
  - Finalize per window: agg @ W (bf16 matmul), scale by dinv(t), exact
    mish via z * a / (a + 2) with a = u^2 + 2u, u = exp(z).

Host side does index work only: bincount (degree), sorting/partitioning,
one-hot packing, int casts, layout packing.
"""

import numpy as np

N_NODES = 100000
IN_CH = 128
N_CORES = 8
TPC = 12500              # targets per core
WIN = 256                # window width (positions); 250 real targets + pad
TPW = 250                # real targets per window
NW = TPC // TPW          # 50 windows
TPAD = NW * WIN          # 12800 output rows per core (window-major positions)
NCHUNK = 4
CHUNK = 25088            # rows per source chunk (196 blocks of 128)
NBLK = CHUNK // 128      # 196
NROWS_PAD = NCHUNK * CHUNK
GU_MAX = 48              # subtiles (128 idx each) per dma_gather unit
OB_BATCH = 16            # one-hot subtiles per HBM load
NQ = 1                   # SWDGE queues


def _build_schedule(row, col):
    """Partition edges; balance windows per core; return host arrays.

    row/col: int64 [E_tot] including self-loops.
    SPMD: slot sizes (s_slot) are shared across cores (max over cores).
    """
    # strided target sharding (core = t % 8) so each core's self-loops
    # spread uniformly over source chunks (keeps slot sizes balanced)
    core = col % N_CORES
    ch = row // CHUNK
    srcloc = (row - ch * CHUNK).astype(np.int64)

    # per-target in-edge counts -> serpentine window assignment per core
    tcnt = np.bincount(col, minlength=N_NODES)
    wassign = np.empty(N_NODES, np.int64)
    wpos = np.empty(N_NODES, np.int64)
    for cc in range(N_CORES):
        tgts = np.arange(cc, N_NODES, N_CORES)
        rank = np.argsort(-tcnt[tgts], kind="stable")
        snake = rank.reshape(TPW, NW).copy()   # row k = rank[k*NW:(k+1)*NW]
        snake[1::2] = snake[1::2, ::-1]        # serpentine
        # local target snake[k, j] -> window j, position k
        wa = np.empty(TPC, np.int64)
        wp = np.empty(TPC, np.int64)
        kk, jj = np.divmod(np.arange(TPW * NW), NW)
        wa[snake.ravel()] = jj
        wp[snake.ravel()] = kk
        wassign[tgts] = wa
        wpos[tgts] = wp

    w = wassign[col]
    tl = wpos[col]                             # window-relative position

    key = ((core * NCHUNK + ch) * NW + w).astype(np.int64)
    counts = np.bincount(key, minlength=N_CORES * NCHUNK * NW).reshape(
        N_CORES, NCHUNK, NW
    )
    s_slot = -(-counts.max(axis=0) // 128)     # [NCHUNK, NW] subtiles
    nsub = int(s_slot.sum())

    slots = []                                 # (chunk, window, sub_base, n)
    sub_base = np.zeros((NCHUNK, NW), np.int64)
    n = 0
    for c in range(NCHUNK):
        for ww in range(NW):
            sub_base[c, ww] = n
            if s_slot[c, ww] > 0:
                slots.append((c, ww, n, int(s_slot[c, ww])))
            n += int(s_slot[c, ww])
    assert n == nsub

    units = []                                 # (chunk, sub_base, su, queue)
    qn = 0
    for c in range(NCHUNK):
        lo = int(sub_base[c, 0])
        hi = lo + int(s_slot[c].sum())
        off = lo
        while off < hi:
            su = min(GU_MAX, hi - off)
            units.append((c, off, su, qn % NQ))
            qn += 1
            off += su

    order = np.lexsort((row, w, ch, core))
    so = srcloc[order]
    tlo = tl[order]
    cum = np.zeros((N_CORES, NCHUNK, NW + 1), np.int64)
    cum[:, :, 1:] = np.cumsum(counts, axis=2)
    ccum = np.zeros((N_CORES, NCHUNK + 1), np.int64)
    ccum[:, 1:] = np.cumsum(counts.sum(axis=2), axis=1)
    gcum = np.zeros(N_CORES + 1, np.int64)
    gcum[1:] = np.cumsum(counts.sum(axis=(1, 2)))

    idx16 = np.zeros((N_CORES, 128, nsub * 8), np.int16)
    tloc = np.full((N_CORES, 128, nsub), -1, np.int32)

    for cc in range(N_CORES):
        for c in range(NCHUNK):
            for ww in range(NW):
                a = gcum[cc] + ccum[cc, c] + cum[cc, c, ww]
                b = gcum[cc] + ccum[cc, c] + cum[cc, c, ww + 1]
                cnt = b - a
                if cnt == 0:
                    continue
                sb = int(sub_base[c, ww])
                npad = int(s_slot[c, ww]) * 128
                sv = np.zeros(npad, np.int16)
                tv = np.full(npad, -1, np.int32)
                sv[:cnt] = so[a:b].astype(np.int16)
                tv[:cnt] = tlo[a:b].astype(np.int32)
                wrapped = sv.reshape(-1, 16).T       # [16, npad/16]
                idx16[cc, :, sb * 8: sb * 8 + npad // 16] = np.tile(
                    wrapped, (8, 1))
                tloc[cc, :, sb: sb + npad // 128] = tv.reshape(-1, 128).T

    # output permutation: per-core window-major position -> global target id
    tperm = np.full((N_CORES, TPAD), -1, np.int64)
    for cc in range(N_CORES):
        tgts = np.arange(cc, N_NODES, N_CORES)
        pos = wassign[tgts] * WIN + wpos[tgts]
        tperm[cc, pos] = tgts

    return idx16, tloc, slots, units, nsub, tperm


def _fix_act_table_loads(nc):
    """All activations used (Ln, Exp, Copy-family) live in the single
    'natural_log_exp_and_others' table set; retarget every load there and
    drop redundant repeats (keep the first load per basic block)."""
    import concourse.mybir as mybir
    from concourse.hw_specs import get_activation_tables

    tables = get_activation_tables(nc.m.arch)
    names = list(tables.keys())
    target = "natural_log_exp_and_others"
    target_id = names.index(target)
    allowed = tables[target]
    for f in nc.m.functions:
        for blk in f.blocks:
            insts = blk.instructions
            for inst in insts:
                if isinstance(inst, mybir.InstActivation):
                    assert inst.func in allowed, inst.func
            kept = []
            seen_load = False
            for inst in insts:
                if isinstance(inst, mybir.InstLoadActFuncSet):
                    si = inst.sync_info
                    has_sync = si is not None and (si.on_wait or si.on_update)
                    if seen_load and not has_sync:
                        continue
                    inst.act_func_set_id = target_id
                    seen_load = True
                kept.append(inst)
            if len(kept) != len(insts):
                insts[:] = kept


def _build_bass(slots, units, nsub):
    import concourse.bacc as bacc
    import concourse.mybir as mybir
    from concourse.tile import TileContext

    AF = mybir.ActivationFunctionType
    OP = mybir.AluOpType
    BF = mybir.dt.bfloat16
    F32 = mybir.dt.float32

    nc = bacc.Bacc("TRN2", target_bir_lowering=False, debug=False,
                   num_devices=N_CORES, num_swdge_queues=NQ,
                   dynamic_dma_scratch_size=32768)
    _orig_compile = nc.compile

    def _compile_with_fix():
        _orig_compile()
        _fix_act_table_loads(nc)

    nc.compile = _compile_with_fix

    xbf_d = nc.dram_tensor("xbf", [NROWS_PAD, IN_CH], BF, kind="ExternalInput")
    idx_d = nc.dram_tensor("idx16", [128, nsub * 8], mybir.dt.int16,
                           kind="ExternalInput")
    ob_d = nc.dram_tensor("obig", [128, nsub * WIN], mybir.dt.float8e4,
                          kind="ExternalInput")
    w_d = nc.dram_tensor("Wb", [IN_CH, IN_CH], BF, kind="ExternalInput")
    degt_d = nc.dram_tensor("degt", [128, TPAD // 128], F32,
                            kind="ExternalInput")
    degs_d = nc.dram_tensor("degs", [128, NROWS_PAD // 128], F32,
                            kind="ExternalInput")
    out_d = nc.dram_tensor("out", [TPAD, IN_CH], F32, kind="ExternalOutput")

    units_by_c = [[] for _ in range(NCHUNK)]
    for u in units:
        units_by_c[u[0]].append(u)
    slots_by_c = [[] for _ in range(NCHUNK)]
    for s in slots:
        slots_by_c[s[0]].append(s)

    with TileContext(nc) as tc:
        with (
            tc.tile_pool(name="const", bufs=1) as cp,
            tc.tile_pool(name="chunk", bufs=1) as chp,
            tc.tile_pool(name="idxp", bufs=2) as ixp,
            tc.tile_pool(name="acc", bufs=1) as ap_,
            tc.tile_pool(name="gt", bufs=4) as gp,
            tc.tile_pool(name="obuf", bufs=3) as op_,
            tc.tile_pool(name="fin", bufs=2) as fp,
            tc.tile_pool(name="xs", bufs=1, space="DRAM") as xsp,
            tc.tile_pool(name="psw", bufs=3, space="PSUM") as pwp,
            tc.tile_pool(name="ps2", bufs=2, space="PSUM") as p2p,
        ):
            wsb = cp.tile([IN_CH, IN_CH], BF)
            nc.sync.dma_start(out=wsb[:], in_=w_d[:])
            degt = cp.tile([128, TPAD // 128], F32)
            nc.sync.dma_start(out=degt[:], in_=degt_d[:])
            degs = cp.tile([128, NROWS_PAD // 128], F32)
            nc.sync.dma_start(out=degs[:], in_=degs_d[:])

            # dinv = exp(-0.5 ln(deg)) for targets and sources
            lnt = cp.tile([128, TPAD // 128], F32)
            nc.scalar.activation(lnt[:], degt[:], AF.Ln)
            dinvt = cp.tile([128, TPAD // 128], F32)
            nc.scalar.activation(dinvt[:], lnt[:], AF.Exp, scale=-0.5)
            lns = cp.tile([128, NROWS_PAD // 128], F32)
            nc.scalar.activation(lns[:], degs[:], AF.Ln)
            dinvs = cp.tile([128, NROWS_PAD // 128], F32)
            nc.scalar.activation(dinvs[:], lns[:], AF.Exp, scale=-0.5)

            accs = [ap_.tile([128, WIN], BF, tag=f"acc{w}", name=f"acc{w}")
                    for w in range(NW)]
            touched = {s[1] for s in slots}
            for w in range(NW):
                if w not in touched:
                    nc.vector.memset(accs[w][:], 0.0)
            first_done = [False] * NW

            last_chunk_of = {}
            for (c_, ww_, _, _) in slots:
                last_chunk_of[ww_] = max(last_chunk_of.get(ww_, -1), c_)
            finalized = set()

            def _finalize_window(w):
                # out[t,:] = mish(dinv_t * (agg[t,:] @ W))
                finalized.add(w)
                for sl in range(WIN // 128):
                    t0 = w * WIN + sl * 128
                    ps2 = p2p.tile([128, 128], F32, tag="ps2", space="PSUM",
                                   name=f"ps2_{w}_{sl}")
                    nc.tensor.matmul(
                        out=ps2[:], lhsT=accs[w][:, sl * 128:(sl + 1) * 128],
                        rhs=wsb[:], start=True, stop=True)
                    zt = fp.tile([128, 128], F32, tag="zt", name=f"zt{w}{sl}")
                    nc.scalar.activation(
                        zt[:], ps2[:], AF.Copy,
                        scale=dinvt[:, 2 * w + sl: 2 * w + sl + 1])
                    u = fp.tile([128, 128], F32, tag="u", name=f"u{w}{sl}")
                    nc.scalar.activation(u[:], zt[:], AF.Exp)
                    a1 = fp.tile([128, 128], F32, tag="a1", name=f"a1{w}{sl}")
                    nc.scalar.activation(a1[:], u[:], AF.Copy, bias=2.0)
                    a = fp.tile([128, 128], F32, tag="a", name=f"a{w}{sl}")
                    nc.vector.tensor_tensor(out=a[:], in0=a1[:], in1=u[:],
                                            op=OP.mult)
                    den = fp.tile([128, 128], F32, tag="den",
                                  name=f"den{w}{sl}")
                    nc.scalar.activation(den[:], a[:], AF.Copy, bias=2.0)
                    rden = fp.tile([128, 128], F32, tag="rden",
                                   name=f"rden{w}{sl}")
                    nc.vector.reciprocal_approx_fast(out=rden[:], in_=den[:])
                    mt = fp.tile([128, 128], F32, tag="mt", name=f"mt{w}{sl}")
                    nc.vector.tensor_tensor(out=mt[:], in0=a[:], in1=rden[:],
                                            op=OP.mult)
                    mz = fp.tile([128, 128], F32, tag="mz", name=f"mz{w}{sl}")
                    nc.vector.tensor_tensor(out=mz[:], in0=mt[:], in1=zt[:],
                                            op=OP.mult)
                    nc.sync.dma_start(out=out_d[t0:t0 + 128, :], in_=mz[:])

            # prologue: build the prescaled bf16 source table xs in DRAM
            xs = [xsp.tile([CHUNK, IN_CH], BF, tag=f"xs{c}", name=f"xs{c}",
                           space="DRAM") for c in range(NCHUNK)]
            HB = NBLK // 2
            for c in range(NCHUNK):
                chunkt = chp.tile([128, NBLK, IN_CH], BF, tag="chunk")
                for h in range(2):
                    b0, b1 = h * HB, (h + 1) * HB if h == 0 else NBLK
                    r0, r1 = c * CHUNK + b0 * 128, c * CHUNK + b1 * 128
                    src = xbf_d[r0:r1, :].rearrange("(b p) f -> p b f", p=128)
                    nc.sync.dma_start(out=chunkt[:, b0:b1, :], in_=src)
                    for b in range(b0, b1):
                        sc = dinvs[:, c * NBLK + b: c * NBLK + b + 1]
                        if b % 2 == 0:
                            nc.scalar.activation(
                                chunkt[:, b, :], chunkt[:, b, :], AF.Copy,
                                scale=sc)
                        else:
                            nc.vector.tensor_scalar(
                                out=chunkt[:, b, :], in0=chunkt[:, b, :],
                                scalar1=sc, scalar2=None, op0=OP.mult)
                    nc.sync.dma_start(
                        out=xs[c][b0 * 128:b1 * 128, :].rearrange(
                            "(b p) f -> p b f", p=128),
                        in_=chunkt[:, b0:b1, :])

            sub_base_c = {}
            off = 0
            for c in range(NCHUNK):
                nsub_c = sum(u[2] for u in units_by_c[c])
                sub_base_c[c] = (off, nsub_c)
                off += nsub_c

            for c in range(NCHUNK):
                base_c, nsub_c = sub_base_c[c]
                idxt = ixp.tile([128, nsub_c * 8], mybir.dt.int16, tag="idxt")
                nc.sync.dma_start(
                    out=idxt[:],
                    in_=idx_d[:, base_c * 8:(base_c + nsub_c) * 8])

                slot_iter = iter(slots_by_c[c])
                cur_slot = next(slot_iter)
                mm_in_slot = 0

                for (_, sb, su, qn) in units_by_c[c]:
                    gT = gp.tile([128, GU_MAX, 128], BF, tag="gt")
                    nc.gpsimd.dma_gather(
                        out_ap=gT[:, :su, :],
                        in_ap=xs[c][:, :],
                        idxs_ap=idxt[:, (sb - base_c) * 8:
                                     (sb - base_c + su) * 8],
                        num_idxs=su * 128,
                        num_idxs_reg=su * 128,
                        elem_size=IN_CH,
                        transpose=False,
                        single_packet=False,
                        queue_num=qn,
                    )
                    for k in range(su):
                        j = sb + k       # global subtile index
                        if k % OB_BATCH == 0:
                            nob = min(OB_BATCH, su - k)
                            ob = op_.tile([128, OB_BATCH * WIN], mybir.dt.float8e4,
                                          tag="ob")
                            nc.sync.dma_start(
                                out=ob[:, :nob * WIN],
                                in_=ob_d[:, j * WIN:(j + nob) * WIN])
                        while j >= cur_slot[2] + cur_slot[3]:
                            cur_slot = next(slot_iter)
                            mm_in_slot = 0
                        (_, ww, ssb, ssz) = cur_slot
                        if mm_in_slot == 0:
                            psw = pwp.tile([128, WIN], F32, tag="psw",
                                           space="PSUM")
                            cur_psw = psw
                        nc.tensor.matmul(
                            out=cur_psw[:], lhsT=gT[:, k, :],
                            rhs=ob[:, (k % OB_BATCH) * WIN:
                                   (k % OB_BATCH + 1) * WIN],
                            start=(mm_in_slot == 0),
                            stop=(mm_in_slot == ssz - 1))
                        mm_in_slot += 1
                        if mm_in_slot == ssz:
                            if not first_done[ww]:
                                first_done[ww] = True
                                nc.scalar.activation(accs[ww][:], cur_psw[:],
                                                     AF.Copy)
                            else:
                                nc.vector.tensor_tensor(
                                    out=accs[ww][:], in0=accs[ww][:],
                                    in1=cur_psw[:], op=OP.add)
                            if c == last_chunk_of[ww]:
                                _finalize_window(ww)

            for w in range(NW):
                if w not in finalized:
                    _finalize_window(w)
    nc.finalize()
    return nc


class _Runner:
    """PJRT runner (axon): jit once, device-resident inputs, reusable."""

    def __init__(self, nc):
        import jax
        import concourse.mybir as mybir
        from jax.sharding import Mesh, PartitionSpec
        from jax.experimental.shard_map import shard_map
        from concourse import bass2jax
        from concourse.bass2jax import _bass_exec_p, install_neuronx_cc_hook

        install_neuronx_cc_hook()
        self.nc = nc
        partition_name = (
            nc.partition_id_tensor.name if nc.partition_id_tensor else None
        )
        in_names, out_names, out_avals, zero_outs = [], [], [], []
        for alloc in nc.m.functions[0].allocations:
            if not isinstance(alloc, mybir.MemoryLocationSet):
                continue
            name = alloc.memorylocations[0].name
            if alloc.kind == "ExternalInput":
                if name != partition_name:
                    in_names.append(name)
            elif alloc.kind == "ExternalOutput":
                shape = tuple(alloc.tensor_shape)
                dtype = mybir.dt.np(alloc.dtype)
                out_names.append(name)
                out_avals.append(jax.core.ShapedArray(shape, dtype))
                zero_outs.append(np.zeros(shape, dtype))
        self.in_names, self.out_names = in_names, out_names
        all_in = list(in_names) + list(out_names)
        if partition_name is not None:
            all_in.append(partition_name)

        def _body(*args):
            operands = list(args)
            if partition_name is not None:
                operands.append(bass2jax.partition_id_tensor())
            return tuple(_bass_exec_p.bind(
                *operands,
                out_avals=tuple(out_avals),
                in_names=tuple(all_in),
                out_names=tuple(out_names),
                lowering_input_output_aliases=(),
                sim_require_finite=True,
                sim_require_nnan=True,
                nc=nc,
            ))

        devices = jax.devices()[:N_CORES]
        mesh = Mesh(np.asarray(devices), ("core",))
        n_in = len(in_names) + len(out_names)
        self.fn = jax.jit(
            shard_map(_body, mesh=mesh,
                      in_specs=(PartitionSpec("core"),) * n_in,
                      out_specs=(PartitionSpec("core"),) * len(out_names),
                      check_rep=False),
            keep_unused=True)
        self.zero_outs = zero_outs
        self.jax = jax

    def stage(self, in_maps):
        args = []
        for name in self.in_names:
            args.append(np.concatenate(
                [np.asarray(m[name]) for m in in_maps], axis=0))
        for z in self.zero_outs:
            args.append(np.concatenate([z] * N_CORES, axis=0))
        self._dev_args = [self.jax.device_put(a) for a in args]
        for a in self._dev_args:
            a.block_until_ready()

    def run(self):
        outs = self.fn(*self._dev_args)
        for o in outs:
            o.block_until_ready()
        return outs

    def results(self, outs):
        per_core = [dict() for _ in range(N_CORES)]
        for i, name in enumerate(self.out_names):
            arr = np.asarray(outs[i])
            for c, piece in enumerate(np.split(arr, N_CORES, axis=0)):
                per_core[c][name] = piece
        return per_core


_CACHE = {}


def _build_inputs(x, W, deg, idx16, tloc, tperm, nsub):
    import ml_dtypes

    N = x.shape[0]
    xbf = np.zeros((NROWS_PAD, IN_CH), ml_dtypes.bfloat16)
    xbf[:N] = x.astype(ml_dtypes.bfloat16)

    # deg value of node n at [n%128, n//128] (pad rows: 1)
    degs = np.ones(NROWS_PAD, np.float32)
    degs[:N] = deg.astype(np.float32)
    degs = degs.reshape(-1, 128).T.copy()        # [128, NROWS_PAD//128]

    Wb = W.astype(ml_dtypes.bfloat16)

    in_maps = []
    for c in range(N_CORES):
        # one-hot scatter tiles: [128, nsub*WIN], O[p, j*WIN + tloc] = 1
        ob = np.zeros((128, nsub * WIN), ml_dtypes.float8_e4m3)
        tl = tloc[c]                              # [128, nsub] int32
        p_, j_ = np.nonzero(tl >= 0)
        ob[p_, j_ * WIN + tl[p_, j_]] = 1

        dp = np.ones(TPAD, np.float32)
        valid = tperm[c] >= 0
        dp[valid] = deg[tperm[c][valid]].astype(np.float32)
        degt = dp.reshape(-1, 128).T.copy()

        in_maps.append({
            "xbf": xbf,
            "idx16": np.ascontiguousarray(idx16[c]),
            "obig": ob,
            "Wb": Wb,
            "degt": np.ascontiguousarray(degt),
            "degs": degs,
        })
    return in_maps


def _prepare(x, edge_index, W, b):
    x = np.asarray(x, dtype=np.float32)
    edge_index = np.asarray(edge_index)
    W = np.asarray(W, dtype=np.float32)
    b = np.asarray(b, dtype=np.float32)
    N = x.shape[0]
    assert N == N_NODES and x.shape[1] == IN_CH
    assert not np.any(b != 0), "bias path not implemented (b is zero here)"

    row = edge_index[0].astype(np.int64)
    col = edge_index[1].astype(np.int64)
    loops = np.arange(N, dtype=np.int64)
    row_all = np.concatenate([row, loops])
    col_all = np.concatenate([col, loops])

    deg = np.bincount(col_all, minlength=N).astype(np.int64)

    idx16, tloc, slots, units, nsub, tperm = _build_schedule(row_all, col_all)
    in_maps = _build_inputs(x, W, deg, idx16, tloc, tperm, nsub)

    key = (nsub, tuple(units), tuple(slots))
    if key not in _CACHE:
        nc = _build_bass(slots, units, nsub)
        runner = _Runner(nc)
        _CACHE.clear()
        _CACHE[key] = runner
    return _CACHE[key], in_maps, tperm


def kernel(x, edge_index, W, b):
    runner, in_maps, tperm = _prepare(x, edge_index, W, b)
    runner.stage(in_maps)
    outs = runner.run()
    res = runner.results(outs)
    full = np.empty((N_NODES, IN_CH), np.float32)
    for c in range(N_CORES):
        valid = tperm[c] >= 0
        full[tperm[c][valid]] = res[c]["out"][valid]
    return full
